# revision 1
# baseline (speedup 1.0000x reference)
"""Causal self-attention (B=2, K=2048, C=1024, H=16) on 8 TRN2 NeuronCores.

Sharding: core c handles batch b = c // 4 and head group g = c % 4
(4 heads = 256 channels). Each core computes qkv projection for its
heads, causal flash-style attention, and a partial output projection
(rows of W_proj for its heads); the host sums the 4 partials per batch
and adds b_proj.

Device layout (per core):
  - inputs pre-transposed/augmented on host: xT_aug [1152, 2048]
    (x[b].T padded with a ones row at 1024 to fold b_qkv), wqk [1152, 512]
    (q|k columns for the 4 heads + bias row), wv [1152, 256], wp [256, 1024].
  - qT/kT computed as [128, 2048] tiles (head pair per tile, Dh=64 on
    partitions), v as [tokens, 256].
  - scores^T per k-tile: row-tiled matmul pair (two heads concurrently,
    K=64 contraction at array rows 0-63 / 64-127) -> s_pair psum
    [128, 1024] (head A cols 0:512, head B 512:1024).
  - causal masks added on PSUM (additive -1e30), one exp (scale=1/8) per
    k-tile over both heads -> p_pair [128, 1024] f32r.
  - pv: M=65 matmuls per head (v extended with a ones column, so psum row
    64 accumulates the softmax denominator), accumulated over k-tiles.
    PSUM matmul outputs must start at partition 0, so each head gets its
    own bank; the head-pair stacking into y_sb [128, SEQ] happens via
    cross-quadrant DVE writes (nch<=64 ops may write either half).
  - denominator rows: DVE reciprocal [1, 512] -> broadcast across 64
    partitions via a K=1 PE matmul -> fused multiply during evacuation.
  - projection: K=128 contraction tiles (head pairs), partial output
    [2048, 1024] DMA'd out.

All matmuls run in float32r (full PE rate for free dim >= 256); operands
are rounded to f32r by the producing instruction (casting DMA or engine
output dtype).
"""

import os

os.environ.setdefault("JAX_PLATFORMS", "axon")

from contextlib import ExitStack

import numpy as np

N_CORES = 8
B, SEQ, C = 2, 2048, 1024
H, DH = 16, 64
CAUG = 1024  # contraction (q/k bias added during evacuation; v bias via K=1 matmul)
NKT = CAUG // 128  # 8
NQC = SEQ // 512  # q-chunks of 512
NTB = SEQ // 128  # 16 token blocks

_CACHE = {}


def _pack_f32r(x):
    """Round fp32 -> fp32r bit pattern (low 12 mantissa bits, round to
    nearest even), returned as a float32-typed array of the packed bits."""
    b = np.ascontiguousarray(x, dtype=np.float32).view(np.uint32)
    low = b & np.uint32(0xFFF)
    base = b & ~np.uint32(0xFFF)
    lsb = (b >> np.uint32(12)) & np.uint32(1)
    up = (low > 0x800) | ((low == 0x800) & (lsb == 1))
    return (base + (up.astype(np.uint32) << np.uint32(12))).view(np.float32)


def _build():
    import concourse.bacc as bacc
    import concourse.mybir as mybir
    import concourse.tile as tile

    F32 = mybir.dt.float32
    F32R = mybir.dt.float32r
    EXP = mybir.ActivationFunctionType.Exp

    nc = bacc.Bacc("TRN2", target_bir_lowering=False, debug=False)

    xT = nc.dram_tensor("xT", [CAUG, SEQ], F32R, kind="ExternalInput").ap()
    wqk = nc.dram_tensor("wqk", [CAUG, 512], F32R, kind="ExternalInput").ap()
    wv = nc.dram_tensor("wv", [CAUG, 256], F32R, kind="ExternalInput").ap()
    wp = nc.dram_tensor("wp", [256, 1024], F32R, kind="ExternalInput").ap()
    ones = nc.dram_tensor("ones", [128, 128], F32R, kind="ExternalInput").ap()
    bqk = nc.dram_tensor("bqk", [128, 4], F32, kind="ExternalInput").ap()
    bv = nc.dram_tensor("bv", [1, 256], F32R, kind="ExternalInput").ap()
    vones = nc.dram_tensor("vones", [128, 64, 1], F32R, kind="ExternalInput").ap()
    mtri = nc.dram_tensor("mtri", [128, 128], F32R, kind="ExternalInput").ap()
    mz3 = nc.dram_tensor("mz3", [128, 256], F32R, kind="ExternalInput").ap()
    out = nc.dram_tensor("out", [SEQ, C], F32, kind="ExternalOutput").ap()

    with tile.TileContext(nc) as tc, ExitStack() as ctx:
        const = ctx.enter_context(tc.tile_pool(name="const", bufs=1))
        wpool = ctx.enter_context(tc.tile_pool(name="wpool", bufs=1))
        qkpool = ctx.enter_context(tc.tile_pool(name="qkpool", bufs=1))
        vpool = ctx.enter_context(tc.tile_pool(name="vpool", bufs=1))
        ypool = ctx.enter_context(tc.tile_pool(name="ypool", bufs=1))

        wqk_sb = []
        wv_sb = []
        wp_sb = []

        # qT[hp], kT[hp]: [128, SEQ] f32r, partitions = head pair channels
        qT = [qkpool.tile([128, SEQ], F32R, name=f"qT{hp}") for hp in range(2)]
        kT = [qkpool.tile([128, SEQ], F32R, name=f"kT{hp}") for hp in range(2)]
        # v extended with a ones column per head: per token block t, head h
        # occupies columns [260 t + 65 h, 260 t + 65 h + 65), col 64 = 1.0
        v_sb = vpool.tile([128, 260 * NTB], F32R, name="v_sb")
        y_sb = [ypool.tile([128, SEQ], F32R, name=f"y{hp}") for hp in range(2)]

        bqk_sb = const.tile([128, 4], F32, name="bqk_sb")
        nc.gpsimd.dma_start(bqk_sb[:], bqk)
        mtri_sb = const.tile([128, 128], F32R, name="mtri_sb")
        nc.gpsimd.dma_start(mtri_sb[:], mtri)
        mz3_sb = const.tile([128, 256], F32R, name="mz3_sb")
        nc.gpsimd.dma_start(mz3_sb[:], mz3)

        def emit_secondary_loads(stage):
            if stage == 1:
                for kk in range(NKT):
                    t = wpool.tile([128, 256], F32R, name=f"wv{kk}")
                    nc.gpsimd.dma_start(t[:], wv[128 * kk : 128 * (kk + 1), :])
                    wv_sb.append(t)
                ones_sb2 = const.tile([128, 128], F32R, name="ones_sb")
                nc.gpsimd.dma_start(ones_sb2[:], ones)
                bv_sb2 = const.tile([1, 256], F32R, name="bv_sb")
                nc.gpsimd.dma_start(bv_sb2[:], bv)
                nc.gpsimd.dma_start(
                    v_sb[:].rearrange("p (b c) -> p b c", c=65)[:, :, 64:65],
                    vones,
                )
                return ones_sb2, bv_sb2

        # --- v projection machinery (dripped into phase 1 and attention) ---
        xv_tiles = {}
        vps_tiles = {}
        pending_v = []  # (t, step) with step NKT == bias matmul

        def emit_xv_loads(j):
            for kk in range(NKT):
                xt = xvp.tile([128, 512], F32R, name=f"xv{j}_{kk}", tag="xv")
                nc.gpsimd.dma_start(
                    xt[:], xT[128 * kk : 128 * (kk + 1), 512 * j : 512 * (j + 1)]
                )
                xv_tiles[(j, kk)] = xt

        def queue_v_block(j, tb):
            for step in range(NKT + 1):
                pending_v.append((4 * j + tb, step))

        def emit_v_steps(n):
            while n > 0 and pending_v:
                t, step = pending_v.pop(0)
                j, tb = t // 4, t % 4
                if step == 0:
                    vps_tiles[t] = psv.tile(
                        [128, 256], F32, name=f"vps{t}", tag="vps"
                    )
                vps = vps_tiles[t]
                if step < NKT:
                    nc.tensor.matmul(
                        vps[:],
                        xv_tiles[(j, step)][:, 128 * tb : 128 * (tb + 1)],
                        wv_sb[step][:],
                        start=(step == 0),
                        stop=False,
                    )
                else:
                    nc.tensor.matmul(
                        vps[:], ones_sb[0:1, :], bv_sb[:], start=False, stop=True
                    )
                    nc.vector.tensor_copy(
                        v_sb[:, 260 * t : 260 * (t + 1)].rearrange(
                            "p (h c) -> p h c", c=65
                        )[:, :, 0:64],
                        vps_tiles.pop(t)[:].rearrange("p (h c) -> p h c", c=64),
                    )
                n -= 1

        def flush_v_through(block):
            while pending_v and pending_v[0][0] <= block:
                emit_v_steps(1)

        # ---------------- phase 1: q/k projection ----------------
        # xT is streamed twice (here per (kk, c4) chunk as the moving operand;
        # again during attention as the stationary operand of the v matmuls)
        # to keep SBUF small. Loads are emitted in consumption order.
        psv = ctx.enter_context(tc.tile_pool(name="psv", bufs=2, space="PSUM"))
        xvp = ctx.enter_context(tc.tile_pool(name="xvp", bufs=20))
        ppool = ctx.enter_context(tc.tile_pool(name="ppool", bufs=5))
        hoisted_p = {}
        with (
            tc.tile_pool(name="xqk", bufs=14) as xqk,
            tc.tile_pool(name="psqk", bufs=6, space="PSUM") as psqk,
        ):
            for tb in range(4):
                queue_v_block(0, tb)
            for c4 in range(NQC):
                cs = slice(512 * c4, 512 * (c4 + 1))
                ps = [
                    psqk.tile([128, 512], F32, name=f"qkps{c4}_{m}", tag="qkps")
                    for m in range(4)
                ]
                for kk in range(NKT):
                    if c4 == 0:
                        t = wpool.tile([128, 512], F32R, name=f"wqk{kk}")
                        nc.sync.dma_start(t[:], wqk[128 * kk : 128 * (kk + 1), :])
                        wqk_sb.append(t)
                    xt = xqk.tile([128, 512], F32R, name=f"x{c4}_{kk}", tag="xqk")
                    eng = nc.sync if kk % 2 == 0 else nc.gpsimd
                    eng.dma_start(xt[:], xT[128 * kk : 128 * (kk + 1), cs])
                    for m in range(4):
                        nc.tensor.matmul(
                            ps[m][:],
                            wqk_sb[kk][:, 128 * m : 128 * (m + 1)],
                            xt[:],
                            start=(kk == 0),
                            stop=(kk == NKT - 1),
                        )
                    if c4 == 3:
                        emit_v_steps(2)
                for m in range(4):
                    dst = qT[m][:, cs] if m < 2 else kT[m - 2][:, cs]
                    nc.scalar.activation(
                        dst, ps[m][:], mybir.ActivationFunctionType.Identity,
                        bias=bqk_sb[:, m : m + 1],
                    )
                if c4 == 1:
                    ones_sb, bv_sb = emit_secondary_loads(1)
                    emit_xv_loads(0)
                if c4 == 0:
                    # hoist chunk (hp=0, j=0): scores + exp start here so the
                    # scalar engine ramps ~40us earlier; pv runs later in the
                    # main attention stream.
                    for hi in range(4):
                        lo = 128 * hi if hi <= 2 else 256
                        w = 512 - lo
                        p = ppool.tile([128, 1024], F32R, name="p_pair")
                        for par in range(2):
                            sps = psqk.tile(
                                [128, 512], F32, name=f"hs{hi}_{par}", tag="qkps"
                            )
                            nc.tensor.matmul(
                                sps[:, lo:512],
                                kT[0][64 * par : 64 * (par + 1),
                                      128 * hi : 128 * (hi + 1)],
                                qT[0][64 * par : 64 * (par + 1), lo:512],
                                start=True,
                                stop=True,
                            )
                            nc.scalar.activation(
                                p[:, 512 * par + lo : 512 * (par + 1)],
                                sps[:, lo:512],
                                EXP,
                                scale=0.125,
                            )
                        if hi <= 2:
                            off, w2, msk = 128 * hi, 128, mtri_sb
                        else:
                            off, w2, msk = 256, 256, mz3_sb
                        pv2 = p[:].rearrange("pt (a q) -> pt a q", q=512)[
                            :, :, off : off + w2
                        ]
                        mv2 = msk[:].rearrange(
                            "pt (a q) -> pt a q", a=1
                        ).broadcast_to([128, 2, w2])
                        nc.vector.tensor_mul(pv2, pv2, mv2)
                        hoisted_p[(0, 0, hi)] = p

        # secondary loads (needed from attention onwards)
        for hp in range(2):
            t = wpool.tile([128, 1024], F32R, name=f"wp{hp}")
            nc.gpsimd.dma_start(t[:], wp[128 * hp : 128 * (hp + 1), :])
            wp_sb.append(t)

        # ------- phase 2: attention (flat software-pipelined stream) -------
        # Items (hp, j, i) are processed in a single pipelined stream: the
        # score matmul pair + exp of item n issue together, the pv matmuls of
        # item n-2 follow, and each chunk's epilogue fires when its last pv
        # has issued. The v projection (hp=0) and the output projection
        # (hp=1) are drip-fed into the stream to fill tensor-engine slack
        # while exp paces the loop.
        with (
            tc.tile_pool(name="pss", bufs=2, space="PSUM") as pss,
            tc.tile_pool(name="psy", bufs=2, space="PSUM") as psy,
            tc.tile_pool(name="epool", bufs=2) as epool,
            tc.tile_pool(name="opool", bufs=4) as opool,
        ):
            o_ps_tiles = {}
            p_tiles = {}

            pending_proj = []
            proj_osb = {}

            def emit_proj_steps(n):
                # one (t, n2) half-block per step: 2 matmuls + evac; the
                # 512 KB store fires when both halves are done
                while n > 0 and pending_proj:
                    t, n2 = pending_proj.pop(0)
                    if n2 == 0:
                        proj_osb[t] = opool.tile(
                            [128, 1024], F32, name=f"po{t}", tag="po"
                        )
                    prps = psv.tile(
                        [128, 512], F32, name=f"prps{t}_{n2}", tag="vps"
                    )
                    for hp2 in range(2):
                        nc.tensor.matmul(
                            prps[:],
                            y_sb[hp2][:, 128 * t : 128 * (t + 1)],
                            wp_sb[hp2][:, 512 * n2 : 512 * (n2 + 1)],
                            start=(hp2 == 0),
                            stop=(hp2 == 1),
                        )
                    if n2 == 0:
                        nc.vector.tensor_copy(
                            proj_osb[t][:, 512 * n2 : 512 * (n2 + 1)], prps[:]
                        )
                    else:
                        nc.scalar.copy(
                            proj_osb[t][:, 512 * n2 : 512 * (n2 + 1)], prps[:]
                        )
                    if n2 == 1:
                        nc.sync.dma_start(
                            out[128 * t : 128 * (t + 1), :], proj_osb.pop(t)[:]
                        )
                    n -= 1

            def queue_proj(j):
                for t in range(4 * j, 4 * j + 4):
                    for n2 in range(2):
                        pending_proj.append((t, n2))

            def emit_s_exp(hp, j, i):
                d = i - 4 * j
                lo = min(max(0, d) * 128, 256)
                s_pair = pss.tile([128, 1024], F32, name="s_pair")
                for par in range(2):
                    nc.tensor.matmul(
                        s_pair[:, 512 * par + lo : 512 * (par + 1)],
                        kT[hp][64 * par : 64 * (par + 1), 128 * i : 128 * (i + 1)],
                        qT[hp][
                            64 * par : 64 * (par + 1),
                            512 * j + lo : 512 * (j + 1),
                        ],
                        start=True,
                        stop=True,
                    )
                p = ppool.tile([128, 1024], F32R, name="p_pair")
                nc.scalar.activation(
                    p[:, lo:1024], s_pair[:, lo:1024], EXP, scale=0.125
                )
                # causal masks: multiplicative 0/1 on p (both heads in one
                # op via a strided view + free-dim broadcast of the mask)
                if d >= 0:
                    if d <= 2:
                        off, w, msk = 128 * d, 128, mtri_sb
                    else:
                        off, w, msk = 256, 256, mz3_sb
                    pv2 = p[:].rearrange("pt (a q) -> pt a q", q=512)[
                        :, :, off : off + w
                    ]
                    mv2 = msk[:].rearrange("pt (a q) -> pt a q", a=1).broadcast_to(
                        [128, 2, w]
                    )
                    nc.vector.tensor_mul(pv2, pv2, mv2)
                p_tiles[(hp, j, i)] = p

            def emit_pv(hp, j, i):
                nk = 4 * j + 4
                lo = min(max(0, i - 4 * j) * 128, 256)
                if hp == 0:
                    flush_v_through(i)
                if i == 0:
                    o_ps_tiles[(hp, j)] = [
                        psy.tile([65, 512], F32, name=f"o_ps{hp}{j}{par}", tag="o_ps")
                        for par in range(2)
                    ]
                o_ps = o_ps_tiles[(hp, j)]
                p = p_tiles.pop((hp, j, i))
                for par in range(2):
                    h = 2 * hp + par
                    vcol = 260 * i + 65 * h
                    nc.tensor.matmul(
                        o_ps[par][:, lo:512],
                        v_sb[:, vcol : vcol + 65],
                        p[:, 512 * par + lo : 512 * (par + 1)],
                        start=(i == 0),
                        stop=(i == nk - 1),
                    )
                if i == nk - 1:
                    emit_epilogue(hp, j)

            def emit_epilogue(hp, j):
                # denominator (row 64) -> broadcast -> fast reciprocal ->
                # fused normalize; decoupled from the main stream once the
                # [65, 512] psum is evacuated (in f32r) to SBUF.
                jc = slice(512 * j, 512 * (j + 1))
                o_ps = o_ps_tiles.pop((hp, j))
                for par in range(2):
                    o_sb = epool.tile([65, 512], F32R, name=f"oe{par}", tag="o_sb")
                    nc.vector.tensor_copy(o_sb[:], o_ps[par][:])
                    bc = psy.tile([64, 512], F32, name=f"bc{par}", tag="o_ps")
                    nc.tensor.matmul(
                        bc[:], ones_sb[64:65, 0:64], o_sb[64:65, :],
                        start=True, stop=True,
                    )
                    r_sb = epool.tile([64, 512], F32, name=f"r_sb{par}", tag="r_sb")
                    nc.vector.reciprocal_approx_fast(out=r_sb[:], in_=bc[:])
                    nc.vector.tensor_mul(
                        y_sb[hp][64 * par : 64 * (par + 1), jc],
                        o_sb[0:64, :],
                        r_sb[:],
                    )
                if hp == 1:
                    queue_proj(j)

            emit_xv_loads(1)

            items = [
                (0, j, i) for j in range(NQC) for i in range(4 * j + 4)
            ] + [
                (1, j, i) for j in (1, 0, 2, 3) for i in range(4 * j + 4)
            ]
            LAG = 3
            for n, (hp, j, i) in enumerate(items):
                if (hp, j, i) in hoisted_p:
                    p_tiles[(hp, j, i)] = hoisted_p.pop((hp, j, i))
                else:
                    emit_s_exp(hp, j, i)
                # drip-fed side work on the tensor engine:
                if hp == 0 and j >= 1:
                    if i == 0:
                        for tb in range(4):
                            queue_v_block(j, tb)
                    emit_v_steps(2)
                if hp == 0 and i == 2 and j < 2:
                    emit_xv_loads(j + 2)
                if hp == 1 and i == 3:
                    emit_proj_steps(8)
                if n >= LAG:
                    emit_pv(*items[n - LAG])
            for n in range(len(items) - LAG, len(items)):
                emit_pv(*items[n])
            emit_proj_steps(len(pending_proj))

    nc.compile()
    return nc


def _get_nc():
    if "nc" not in _CACHE:
        _CACHE["nc"] = _build()
    return _CACHE["nc"]


def _prep_inputs(x, W_qkv, b_qkv, W_proj, b_proj):
    """Build the 8 per-core input maps."""
    x = np.asarray(x, dtype=np.float32)
    W_qkv = np.asarray(W_qkv, dtype=np.float32)
    b_qkv = np.asarray(b_qkv, dtype=np.float32)
    W_proj = np.asarray(W_proj, dtype=np.float32)

    ones = np.ones((128, 128), dtype=np.float32)
    vones = np.ones((128, 64, 1), dtype=np.float32)
    mtri = (np.arange(128)[:, None] <= np.arange(128)[None, :]).astype(np.float32)
    mz3 = np.concatenate(
        [np.zeros((128, 128), dtype=np.float32), mtri], axis=1
    )

    xT_aug = {}
    for b in range(B):
        xT_aug[b] = _pack_f32r(x[b].T)

    in_maps = []
    for c in range(N_CORES):
        b = c // 4
        g = c % 4
        hs = slice(256 * g, 256 * (g + 1))
        wqk = np.concatenate(
            [W_qkv[:, 0:1024][:, hs], W_qkv[:, 1024:2048][:, hs]], axis=1
        )
        wv = W_qkv[:, 2048:3072][:, hs]
        bqk_h = np.stack(
            [b_qkv[0:1024][hs], b_qkv[1024:2048][hs]]
        )  # [2, 256] -> m blocks of 128
        bqk_m = np.concatenate([bqk_h[0], bqk_h[1]]).reshape(4, 128).T.copy()
        bv_h = _pack_f32r(b_qkv[2048:3072][hs].reshape(1, 256))
        wp = _pack_f32r(W_proj[hs, :])
        in_maps.append(
            {
                "xT": xT_aug[b],
                "wqk": _pack_f32r(wqk),
                "wv": _pack_f32r(wv),
                "wp": wp,
                "ones": ones,
                "bqk": np.ascontiguousarray(bqk_m),
                "bv": bv_h,
                "vones": vones,
                "mtri": mtri,
                "mz3": mz3,
            }
        )
    return in_maps


def kernel(x, W_qkv, b_qkv, W_proj, b_proj, K=None, _trace=False):
    from concourse.bass_utils import run_bass_kernel_spmd

    in_maps = _prep_inputs(x, W_qkv, b_qkv, W_proj, b_proj)
    nc = _get_nc()
    res = run_bass_kernel_spmd(
        nc, in_maps, core_ids=list(range(N_CORES)), trace=_trace
    )
    parts = [res.results[c]["out"] for c in range(N_CORES)]
    b_proj = np.asarray(b_proj, dtype=np.float32)
    y = np.empty((B, SEQ, C), dtype=np.float32)
    for b in range(B):
        y[b] = parts[4 * b] + parts[4 * b + 1] + parts[4 * b + 2] + parts[4 * b + 3]
        y[b] += b_proj
    if _trace:
        _CACHE["last_exec_time_ns"] = res.exec_time_ns
        _CACHE["last_results"] = res
    return y



# revision 5
# speedup vs baseline: 1.3346x; 1.3346x over previous
"""Causal self-attention (B=2, K=2048, C=1024, H=16) on 8 TRN2 NeuronCores.

Sharding: core c handles batch b = c // 4 and head group g = c % 4
(4 heads = 256 channels). Each core computes qkv projection for its
heads, causal flash-style attention, and a partial output projection
(rows of W_proj for its heads); the host sums the 4 partials per batch
and adds b_proj.

All matmul operands are bf16 (PE double-pump: 2 cols/cycle, half-size
weight loads) with fp32 PSUM accumulation; elementwise work on p/y runs
at the DVE/Pool 16-bit rate. x is shipped once as bf16 [1024, 2048] and
stays resident in SBUF for both the q/k projection (moving operand) and
the v projection (stationary operand).

Device layout (per core):
  - qT/kT computed as [128, 2048] bf16 tiles (head pair per tile, Dh=64
    on partitions), v as [tokens, 260*16] bf16 with a ones column per
    head (psum row 64 of the pv matmul accumulates the softmax
    denominator).
  - scores^T per k-tile: row-tiled matmul pair (two heads concurrently,
    K=64 contraction at array rows 0-63 / 64-127) -> s_pair psum
    [128, 1024] f32 (head A cols 0:512, head B 512:1024).
  - one exp (scale=1/8) per k-tile over both heads -> p_pair [128, 1024]
    bf16; causal masks are multiplicative 0/1 on p (vector engine).
  - pv accumulated over k-tiles into [65, 512] psum per head; the
    denominator row is broadcast via a K=1 PE matmul, reciprocal on DVE,
    fused normalize during evacuation.
  - projection: K=128 contraction tiles (head pairs), partial output
    [2048, 1024] f32 DMA'd out.
"""

import os

os.environ.setdefault("JAX_PLATFORMS", "axon")

from contextlib import ExitStack

import ml_dtypes
import numpy as np

N_CORES = 8
B, SEQ, C = 2, 2048, 1024
H, DH = 16, 64
CAUG = 1024  # contraction (q/k bias added during evacuation; v bias via K=1 matmul)
NKT = CAUG // 128  # 8
NQC = SEQ // 512  # q-chunks of 512
NTB = SEQ // 128  # 16 token blocks

_CACHE = {}

BF16NP = ml_dtypes.bfloat16


def _bf16(x):
    return np.ascontiguousarray(np.asarray(x, dtype=np.float32).astype(BF16NP))


def _build():
    import concourse.bacc as bacc
    import concourse.mybir as mybir
    import concourse.tile as tile

    F32 = mybir.dt.float32
    BF16 = mybir.dt.bfloat16
    EXP = mybir.ActivationFunctionType.Exp

    nc = bacc.Bacc("TRN2", target_bir_lowering=False, debug=False)

    xT = nc.dram_tensor("xT", [CAUG, SEQ], BF16, kind="ExternalInput").ap()
    wqk = nc.dram_tensor("wqk", [CAUG, 512], BF16, kind="ExternalInput").ap()
    wv = nc.dram_tensor("wv", [CAUG, 256], BF16, kind="ExternalInput").ap()
    wp = nc.dram_tensor("wp", [256, 1024], BF16, kind="ExternalInput").ap()
    ones = nc.dram_tensor("ones", [128, 128], BF16, kind="ExternalInput").ap()
    bqk = nc.dram_tensor("bqk", [128, 4], F32, kind="ExternalInput").ap()
    bv = nc.dram_tensor("bv", [1, 256], BF16, kind="ExternalInput").ap()
    vones = nc.dram_tensor("vones", [128, 64, 1], BF16, kind="ExternalInput").ap()
    mtri = nc.dram_tensor("mtri", [128, 128], BF16, kind="ExternalInput").ap()
    mz3 = nc.dram_tensor("mz3", [128, 256], BF16, kind="ExternalInput").ap()
    out = nc.dram_tensor("out", [SEQ, C], F32, kind="ExternalOutput").ap()

    with tile.TileContext(nc) as tc, ExitStack() as ctx:
        const = ctx.enter_context(tc.tile_pool(name="const", bufs=1))
        wpool = ctx.enter_context(tc.tile_pool(name="wpool", bufs=1))
        qkpool = ctx.enter_context(tc.tile_pool(name="qkpool", bufs=1))
        vpool = ctx.enter_context(tc.tile_pool(name="vpool", bufs=1))
        ypool = ctx.enter_context(tc.tile_pool(name="ypool", bufs=1))
        xpool = ctx.enter_context(tc.tile_pool(name="xpool", bufs=32))

        wqk_sb = []
        wv_sb = []
        wp_sb = []
        x_tiles = {}  # (j, kk) -> [128, 512] bf16, tokens 512j.., rows 128kk..

        # qT[hp], kT[hp]: [128, SEQ] bf16, partitions = head pair channels
        qT = [qkpool.tile([128, SEQ], BF16, name=f"qT{hp}") for hp in range(2)]
        kT = [qkpool.tile([128, SEQ], BF16, name=f"kT{hp}") for hp in range(2)]
        # v extended with a ones column per head: per token block t, head h
        # occupies columns [260 t + 65 h, 260 t + 65 h + 65), col 64 = 1.0
        v_sb = vpool.tile([128, 260 * NTB], BF16, name="v_sb")
        y_sb = [ypool.tile([128, SEQ], BF16, name=f"y{hp}") for hp in range(2)]

        bqk_sb = const.tile([128, 4], F32, name="bqk_sb")
        nc.gpsimd.dma_start(bqk_sb[:], bqk)
        mtri_sb = const.tile([128, 128], BF16, name="mtri_sb")
        nc.gpsimd.dma_start(mtri_sb[:], mtri)
        mz3_sb = const.tile([128, 256], BF16, name="mz3_sb")
        nc.gpsimd.dma_start(mz3_sb[:], mz3)

        def emit_secondary_loads(stage):
            if stage == 1:
                for kk in range(NKT):
                    t = wpool.tile([128, 256], BF16, name=f"wv{kk}")
                    nc.gpsimd.dma_start(t[:], wv[128 * kk : 128 * (kk + 1), :])
                    wv_sb.append(t)
                ones_sb2 = const.tile([128, 128], BF16, name="ones_sb")
                nc.gpsimd.dma_start(ones_sb2[:], ones)
                bv_sb2 = const.tile([1, 256], BF16, name="bv_sb")
                nc.gpsimd.dma_start(bv_sb2[:], bv)
                nc.gpsimd.dma_start(
                    v_sb[:].rearrange("p (b c) -> p b c", c=65)[:, :, 64:65],
                    vones,
                )
                return ones_sb2, bv_sb2

        # --- v projection machinery (dripped into phase 1 and attention) ---
        xv_tiles = {}
        vps_tiles = {}
        pending_v = []  # (t, step) with step NKT == bias matmul

        def emit_xv_loads(j):
            for kk in range(NKT):
                xt = xvp.tile([128, 512], BF16, name=f"xv{j}_{kk}", tag="xv")
                nc.gpsimd.dma_start(
                    xt[:], xT[128 * kk : 128 * (kk + 1), 512 * j : 512 * (j + 1)]
                )
                xv_tiles[(j, kk)] = xt

        def queue_v_block(j, tb):
            for step in range(NKT + 1):
                pending_v.append((4 * j + tb, step))

        def emit_v_steps(n):
            while n > 0 and pending_v:
                t, step = pending_v.pop(0)
                j, tb = t // 4, t % 4
                if step == 0:
                    vps_tiles[t] = psv.tile(
                        [128, 256], F32, name=f"vps{t}", tag="vps"
                    )
                vps = vps_tiles[t]
                if step < NKT:
                    nc.tensor.matmul(
                        vps[:],
                        xv_tiles[(j, step)][:, 128 * tb : 128 * (tb + 1)],
                        wv_sb[step][:],
                        start=(step == 0),
                        stop=False,
                    )
                else:
                    nc.tensor.matmul(
                        vps[:], ones_sb[0:1, :], bv_sb[:], start=False, stop=True
                    )
                    nc.vector.tensor_copy(
                        v_sb[:, 260 * t : 260 * (t + 1)].rearrange(
                            "p (h c) -> p h c", c=65
                        )[:, :, 0:64],
                        vps_tiles.pop(t)[:].rearrange("p (h c) -> p h c", c=64),
                    )
                n -= 1

        def flush_v_through(block):
            while pending_v and pending_v[0][0] <= block:
                emit_v_steps(1)

        # ---------------- phase 1: q/k projection ----------------
        # x tiles stay resident in SBUF (bf16): moving operand here, then
        # stationary operand of the v matmuls. Loads in consumption order.
        psv = ctx.enter_context(tc.tile_pool(name="psv", bufs=2, space="PSUM"))
        xvp = ctx.enter_context(tc.tile_pool(name="xvp", bufs=20))
        ppool = ctx.enter_context(tc.tile_pool(name="ppool", bufs=5))
        hoisted_p = {}
        with tc.tile_pool(name="psqk", bufs=6, space="PSUM") as psqk:
            for tb in range(4):
                queue_v_block(0, tb)
            for c4 in range(NQC):
                cs = slice(512 * c4, 512 * (c4 + 1))
                ps = [
                    psqk.tile([128, 512], F32, name=f"qkps{c4}_{m}", tag="qkps")
                    for m in range(4)
                ]
                for kk in range(NKT):
                    if c4 == 0:
                        t = wpool.tile([128, 512], BF16, name=f"wqk{kk}")
                        nc.sync.dma_start(t[:], wqk[128 * kk : 128 * (kk + 1), :])
                        wqk_sb.append(t)
                    xt = xpool.tile([128, 512], BF16, name=f"x{c4}_{kk}", tag="x")
                    eng = nc.sync if kk % 2 == 0 else nc.gpsimd
                    eng.dma_start(xt[:], xT[128 * kk : 128 * (kk + 1), cs])
                    x_tiles[(c4, kk)] = xt
                    for m in range(4):
                        nc.tensor.matmul(
                            ps[m][:],
                            wqk_sb[kk][:, 128 * m : 128 * (m + 1)],
                            xt[:],
                            start=(kk == 0),
                            stop=(kk == NKT - 1),
                        )
                    if c4 == 3:
                        emit_v_steps(2)
                for m in range(4):
                    dst = qT[m][:, cs] if m < 2 else kT[m - 2][:, cs]
                    nc.scalar.activation(
                        dst, ps[m][:], mybir.ActivationFunctionType.Identity,
                        bias=bqk_sb[:, m : m + 1],
                    )
                if c4 == 1:
                    ones_sb, bv_sb = emit_secondary_loads(1)
                    emit_xv_loads(0)
                if c4 == 0:
                    # hoist chunk (hp=0, j=0): scores + exp start here so the
                    # scalar engine ramps earlier; pv runs later in the
                    # main attention stream.
                    for hi in range(4):
                        lo = 128 * hi if hi <= 2 else 256
                        w = 512 - lo
                        p = ppool.tile([128, 1024], BF16, name="p_pair")
                        for par in range(2):
                            sps = psqk.tile(
                                [128, 512], F32, name=f"hs{hi}_{par}", tag="qkps"
                            )
                            nc.tensor.matmul(
                                sps[:, lo:512],
                                kT[0][64 * par : 64 * (par + 1),
                                      128 * hi : 128 * (hi + 1)],
                                qT[0][64 * par : 64 * (par + 1), lo:512],
                                start=True,
                                stop=True,
                            )
                            nc.scalar.activation(
                                p[:, 512 * par + lo : 512 * (par + 1)],
                                sps[:, lo:512],
                                EXP,
                                scale=0.125,
                            )
                        if hi <= 2:
                            off, w2, msk = 128 * hi, 128, mtri_sb
                        else:
                            off, w2, msk = 256, 256, mz3_sb
                        pv2 = p[:].rearrange("pt (a q) -> pt a q", q=512)[
                            :, :, off : off + w2
                        ]
                        mv2 = msk[:].rearrange(
                            "pt (a q) -> pt a q", a=1
                        ).broadcast_to([128, 2, w2])
                        nc.vector.tensor_mul(pv2, pv2, mv2)
                        hoisted_p[(0, 0, hi)] = p

        # secondary loads (needed from attention onwards)
        for hp in range(2):
            t = wpool.tile([128, 1024], BF16, name=f"wp{hp}")
            nc.gpsimd.dma_start(t[:], wp[128 * hp : 128 * (hp + 1), :])
            wp_sb.append(t)

        # ------- phase 2: attention (flat software-pipelined stream) -------
        # Items (hp, j, i) are processed in a single pipelined stream: the
        # score matmul pair + exp of item n issue together, the pv matmuls of
        # item n-2 follow, and each chunk's epilogue fires when its last pv
        # has issued. The v projection (hp=0) and the output projection
        # (hp=1) are drip-fed into the stream to fill tensor-engine slack
        # while exp paces the loop.
        with (
            tc.tile_pool(name="pss", bufs=2, space="PSUM") as pss,
            tc.tile_pool(name="psy", bufs=2, space="PSUM") as psy,
            tc.tile_pool(name="epool", bufs=2) as epool,
            tc.tile_pool(name="opool", bufs=4) as opool,
        ):
            o_ps_tiles = {}
            p_tiles = {}

            pending_proj = []
            proj_osb = {}

            def emit_proj_steps(n):
                # one (t, n2) half-block per step: 2 matmuls + evac; the
                # 512 KB store fires when both halves are done
                while n > 0 and pending_proj:
                    t, n2 = pending_proj.pop(0)
                    if n2 == 0:
                        proj_osb[t] = opool.tile(
                            [128, 1024], F32, name=f"po{t}", tag="po"
                        )
                    prps = psv.tile(
                        [128, 512], F32, name=f"prps{t}_{n2}", tag="vps"
                    )
                    for hp2 in range(2):
                        nc.tensor.matmul(
                            prps[:],
                            y_sb[hp2][:, 128 * t : 128 * (t + 1)],
                            wp_sb[hp2][:, 512 * n2 : 512 * (n2 + 1)],
                            start=(hp2 == 0),
                            stop=(hp2 == 1),
                        )
                    if n2 == 0:
                        nc.vector.tensor_copy(
                            proj_osb[t][:, 512 * n2 : 512 * (n2 + 1)], prps[:]
                        )
                    else:
                        nc.scalar.copy(
                            proj_osb[t][:, 512 * n2 : 512 * (n2 + 1)], prps[:]
                        )
                    if n2 == 1:
                        nc.sync.dma_start(
                            out[128 * t : 128 * (t + 1), :], proj_osb.pop(t)[:]
                        )
                    n -= 1

            def queue_proj(j):
                for t in range(4 * j, 4 * j + 4):
                    for n2 in range(2):
                        pending_proj.append((t, n2))

            def emit_s_exp(hp, j, i):
                d = i - 4 * j
                lo = min(max(0, d) * 128, 256)
                s_pair = pss.tile([128, 1024], F32, name="s_pair")
                for par in range(2):
                    nc.tensor.matmul(
                        s_pair[:, 512 * par + lo : 512 * (par + 1)],
                        kT[hp][64 * par : 64 * (par + 1), 128 * i : 128 * (i + 1)],
                        qT[hp][
                            64 * par : 64 * (par + 1),
                            512 * j + lo : 512 * (j + 1),
                        ],
                        start=True,
                        stop=True,
                    )
                p = ppool.tile([128, 1024], BF16, name="p_pair")
                nc.scalar.activation(
                    p[:, lo:1024], s_pair[:, lo:1024], EXP, scale=0.125
                )
                # causal masks: multiplicative 0/1 on p (both heads in one
                # op via a strided view + free-dim broadcast of the mask)
                if d >= 0:
                    if d <= 2:
                        off, w, msk = 128 * d, 128, mtri_sb
                    else:
                        off, w, msk = 256, 256, mz3_sb
                    pv2 = p[:].rearrange("pt (a q) -> pt a q", q=512)[
                        :, :, off : off + w
                    ]
                    mv2 = msk[:].rearrange("pt (a q) -> pt a q", a=1).broadcast_to(
                        [128, 2, w]
                    )
                    nc.vector.tensor_mul(pv2, pv2, mv2)
                p_tiles[(hp, j, i)] = p

            def emit_pv(hp, j, i):
                nk = 4 * j + 4
                lo = min(max(0, i - 4 * j) * 128, 256)
                if hp == 0:
                    flush_v_through(i)
                if i == 0:
                    o_ps_tiles[(hp, j)] = [
                        psy.tile([65, 512], F32, name=f"o_ps{hp}{j}{par}", tag="o_ps")
                        for par in range(2)
                    ]
                o_ps = o_ps_tiles[(hp, j)]
                p = p_tiles.pop((hp, j, i))
                for par in range(2):
                    h = 2 * hp + par
                    vcol = 260 * i + 65 * h
                    nc.tensor.matmul(
                        o_ps[par][:, lo:512],
                        v_sb[:, vcol : vcol + 65],
                        p[:, 512 * par + lo : 512 * (par + 1)],
                        start=(i == 0),
                        stop=(i == nk - 1),
                    )
                if i == nk - 1:
                    emit_epilogue(hp, j)

            def emit_epilogue(hp, j):
                # denominator (row 64) -> broadcast -> fast reciprocal ->
                # fused normalize; decoupled from the main stream once the
                # [65, 512] psum is evacuated (in bf16) to SBUF.
                jc = slice(512 * j, 512 * (j + 1))
                o_ps = o_ps_tiles.pop((hp, j))
                for par in range(2):
                    o_sb = epool.tile([65, 512], BF16, name=f"oe{par}", tag="o_sb")
                    nc.vector.tensor_copy(o_sb[:], o_ps[par][:])
                    bc = psy.tile([64, 512], F32, name=f"bc{par}", tag="o_ps")
                    nc.tensor.matmul(
                        bc[:], ones_sb[64:65, 0:64], o_sb[64:65, :],
                        start=True, stop=True,
                    )
                    r_sb = epool.tile([64, 512], F32, name=f"r_sb{par}", tag="r_sb")
                    nc.vector.reciprocal_approx_fast(out=r_sb[:], in_=bc[:])
                    nc.vector.tensor_mul(
                        y_sb[hp][64 * par : 64 * (par + 1), jc],
                        o_sb[0:64, :],
                        r_sb[:],
                    )
                if hp == 1:
                    queue_proj(j)

            emit_xv_loads(1)

            items = [
                (0, j, i) for j in range(NQC) for i in range(4 * j + 4)
            ] + [
                (1, j, i) for j in (1, 0, 2, 3) for i in range(4 * j + 4)
            ]
            LAG = 3
            for n, (hp, j, i) in enumerate(items):
                if (hp, j, i) in hoisted_p:
                    p_tiles[(hp, j, i)] = hoisted_p.pop((hp, j, i))
                else:
                    emit_s_exp(hp, j, i)
                # drip-fed side work on the tensor engine:
                if hp == 0 and j >= 1:
                    if i == 0:
                        for tb in range(4):
                            queue_v_block(j, tb)
                    emit_v_steps(2)
                if hp == 0 and i == 2 and j < 2:
                    emit_xv_loads(j + 2)
                if hp == 1 and i == 3:
                    emit_proj_steps(8)
                if n >= LAG:
                    emit_pv(*items[n - LAG])
            for n in range(len(items) - LAG, len(items)):
                emit_pv(*items[n])
            emit_proj_steps(len(pending_proj))

    nc.compile()
    return nc


def _get_nc():
    if "nc" not in _CACHE:
        _CACHE["nc"] = _build()
    return _CACHE["nc"]


def _prep_inputs(x, W_qkv, b_qkv, W_proj, b_proj):
    """Build the 8 per-core input maps."""
    x = np.asarray(x, dtype=np.float32)
    W_qkv = np.asarray(W_qkv, dtype=np.float32)
    b_qkv = np.asarray(b_qkv, dtype=np.float32)
    W_proj = np.asarray(W_proj, dtype=np.float32)

    ones = np.ones((128, 128), dtype=BF16NP)
    vones = np.ones((128, 64, 1), dtype=BF16NP)
    mtri = (np.arange(128)[:, None] <= np.arange(128)[None, :]).astype(BF16NP)
    mz3 = np.concatenate(
        [np.zeros((128, 128), dtype=BF16NP), np.asarray(mtri)], axis=1
    )

    xT_aug = {}
    for b in range(B):
        xT_aug[b] = _bf16(x[b].T)

    in_maps = []
    for c in range(N_CORES):
        b = c // 4
        g = c % 4
        hs = slice(256 * g, 256 * (g + 1))
        wqk_c = np.concatenate(
            [W_qkv[:, 0:1024][:, hs], W_qkv[:, 1024:2048][:, hs]], axis=1
        )
        wv_c = W_qkv[:, 2048:3072][:, hs]
        bqk_h = np.stack(
            [b_qkv[0:1024][hs], b_qkv[1024:2048][hs]]
        )  # [2, 256] -> m blocks of 128
        bqk_m = np.concatenate([bqk_h[0], bqk_h[1]]).reshape(4, 128).T.copy()
        bv_h = _bf16(b_qkv[2048:3072][hs].reshape(1, 256))
        wp_c = _bf16(W_proj[hs, :])
        in_maps.append(
            {
                "xT": xT_aug[b],
                "wqk": _bf16(wqk_c),
                "wv": _bf16(wv_c),
                "wp": wp_c,
                "ones": ones,
                "bqk": np.ascontiguousarray(bqk_m),
                "bv": bv_h,
                "vones": vones,
                "mtri": np.ascontiguousarray(mtri),
                "mz3": mz3,
            }
        )
    return in_maps


def kernel(x, W_qkv, b_qkv, W_proj, b_proj, K=None, _trace=False):
    from concourse.bass_utils import run_bass_kernel_spmd

    in_maps = _prep_inputs(x, W_qkv, b_qkv, W_proj, b_proj)
    nc = _get_nc()
    res = run_bass_kernel_spmd(
        nc, in_maps, core_ids=list(range(N_CORES)), trace=_trace
    )
    parts = [res.results[c]["out"] for c in range(N_CORES)]
    b_proj = np.asarray(b_proj, dtype=np.float32)
    y = np.empty((B, SEQ, C), dtype=np.float32)
    for b in range(B):
        y[b] = parts[4 * b] + parts[4 * b + 1] + parts[4 * b + 2] + parts[4 * b + 3]
        y[b] += b_proj
    if _trace:
        _CACHE["last_exec_time_ns"] = res.exec_time_ns
        _CACHE["last_results"] = res
    return y


# revision 10
# speedup vs baseline: 1.4878x; 1.1148x over previous
"""Causal self-attention (B=2, K=2048, C=1024, H=16) on 8 TRN2 NeuronCores.

Sharding: core c handles batch b = c // 4 and head group g = c % 4
(4 heads = 256 channels). Each core computes qkv projection for its
heads, causal flash-style attention, and a partial output projection
(rows of W_proj for its heads); the host sums the 4 partials per batch
and adds b_proj.

All matmul operands are bf16 (PE double-pump: 2 cols/cycle, half-size
weight loads) with fp32 PSUM accumulation; elementwise work on p/y runs
at the DVE/Pool 16-bit rate. x is shipped once as bf16 [1024, 2048] and
stays resident in SBUF for both the q/k projection (moving operand) and
the v projection (stationary operand).

Device layout (per core):
  - qT/kT computed as [128, 2048] bf16 tiles (head pair per tile, Dh=64
    on partitions), v as [tokens, 260*16] bf16 with a ones column per
    head (psum row 64 of the pv matmul accumulates the softmax
    denominator).
  - scores^T per k-tile: row-tiled matmul pair (two heads concurrently,
    K=64 contraction at array rows 0-63 / 64-127) -> s_pair psum
    [128, 1024] f32 (head A cols 0:512, head B 512:1024).
  - one exp (scale=1/8) per k-tile over both heads -> p_pair [128, 1024]
    bf16; causal masks are multiplicative 0/1 on p (vector engine).
  - pv accumulated over k-tiles into [65, 512] psum per head; the
    denominator row is broadcast via a K=1 PE matmul, reciprocal on DVE,
    fused normalize during evacuation.
  - projection: K=128 contraction tiles (head pairs), partial output
    [2048, 1024] f32 DMA'd out.
"""

import os

os.environ.setdefault("JAX_PLATFORMS", "axon")

from contextlib import ExitStack

import ml_dtypes
import numpy as np

N_CORES = 8
B, SEQ, C = 2, 2048, 1024
H, DH = 16, 64
CAUG = 1024  # contraction (q/k bias added during evacuation; v bias via K=1 matmul)
NKT = CAUG // 128  # 8
NQC = SEQ // 512  # q-chunks of 512
NTB = SEQ // 128  # 16 token blocks

_CACHE = {}

BF16NP = ml_dtypes.bfloat16
FP8NP = ml_dtypes.float8_e4m3


def _bf16(x):
    return np.ascontiguousarray(np.asarray(x, dtype=np.float32).astype(BF16NP))


def _fp8(x):
    return np.ascontiguousarray(np.asarray(x, dtype=np.float32).astype(FP8NP))


def _build():
    import concourse.bacc as bacc
    import concourse.mybir as mybir
    import concourse.tile as tile

    F32 = mybir.dt.float32
    BF16 = mybir.dt.bfloat16
    FP8 = mybir.dt.float8e4
    DR = mybir.MatmulPerfMode.DoubleRow
    EXP = mybir.ActivationFunctionType.Exp
    IDN = mybir.ActivationFunctionType.Identity

    nc = bacc.Bacc("TRN2", target_bir_lowering=False, debug=False)

    xT = nc.dram_tensor("xT", [CAUG, SEQ], FP8, kind="ExternalInput").ap()
    wqk = nc.dram_tensor("wqk", [CAUG, 512], FP8, kind="ExternalInput").ap()
    xT0b = nc.dram_tensor("xT0b", [CAUG, 512], BF16, kind="ExternalInput").ap()
    wqkb = nc.dram_tensor("wqkb", [CAUG, 512], BF16, kind="ExternalInput").ap()
    wvb = nc.dram_tensor("wvb", [CAUG, 256], BF16, kind="ExternalInput").ap()
    wv = nc.dram_tensor("wv", [CAUG, 256], FP8, kind="ExternalInput").ap()
    wp = nc.dram_tensor("wp", [256, 1024], BF16, kind="ExternalInput").ap()
    ones = nc.dram_tensor("ones", [128, 128], BF16, kind="ExternalInput").ap()
    bqk = nc.dram_tensor("bqk", [128, 4], F32, kind="ExternalInput").ap()
    bv = nc.dram_tensor("bv", [1, 256], BF16, kind="ExternalInput").ap()
    bv16 = nc.dram_tensor("bv16", [1, 256], BF16, kind="ExternalInput").ap()
    vones = nc.dram_tensor("vones", [128, 64, 1], BF16, kind="ExternalInput").ap()
    mtri = nc.dram_tensor("mtri", [128, 128], BF16, kind="ExternalInput").ap()
    out = nc.dram_tensor("out", [SEQ, C], F32, kind="ExternalOutput").ap()

    with tile.TileContext(nc) as tc, ExitStack() as ctx:
        const = ctx.enter_context(tc.tile_pool(name="const", bufs=1))
        wpool = ctx.enter_context(tc.tile_pool(name="wpool", bufs=1))
        qkpool = ctx.enter_context(tc.tile_pool(name="qkpool", bufs=1))
        vpool = ctx.enter_context(tc.tile_pool(name="vpool", bufs=1))
        ypool = ctx.enter_context(tc.tile_pool(name="ypool", bufs=1))
        xpool = ctx.enter_context(tc.tile_pool(name="xpool", bufs=32))

        wqk_sb = []
        wqkb_sb = []
        wv_sb = []
        wvb_sb = []
        wp_sb = []
        x_tiles = {}  # (j, kk) -> [128, 512] bf16, tokens 512j.., rows 128kk..

        # qT[hp], kT[hp]: [128, SEQ] bf16, partitions = head pair channels
        qT = [qkpool.tile([128, SEQ], BF16, name=f"qT{hp}") for hp in range(2)]
        kT = [qkpool.tile([128, SEQ], BF16, name=f"kT{hp}") for hp in range(2)]
        # v extended with a ones column per head: per token block t, head h
        # occupies columns [260 t + 65 h, 260 t + 65 h + 65), col 64 = 1.0
        v_sb = vpool.tile([128, 260 * NTB], BF16, name="v_sb")
        y_sb = [ypool.tile([128, SEQ], BF16, name=f"y{hp}") for hp in range(2)]

        bqk_sb = const.tile([128, 4], F32, name="bqk_sb")
        nc.gpsimd.dma_start(bqk_sb[:], bqk)
        mtri_sb = const.tile([128, 128], BF16, name="mtri_sb")
        nc.gpsimd.dma_start(mtri_sb[:], mtri)

        def emit_secondary_loads(stage):
            if stage == 1:
                for g in range(4):
                    t = wpool.tile([128, 2, 256], FP8, name=f"wv{g}")
                    for s in range(2):
                        nc.gpsimd.dma_start(
                            t[:, s, :], wv[256 * g + 128 * s : 256 * g + 128 * (s + 1), :]
                        )
                    wv_sb.append(t)
                for kk in range(NKT):
                    t = wpool.tile([128, 256], BF16, name=f"wvb{kk}")
                    nc.gpsimd.dma_start(t[:], wvb[128 * kk : 128 * (kk + 1), :])
                    wvb_sb.append(t)
                ones_sb2 = const.tile([128, 128], BF16, name="ones_sb")
                nc.gpsimd.dma_start(ones_sb2[:], ones)
                bv_sb2 = const.tile([1, 256], BF16, name="bv_sb")
                nc.gpsimd.dma_start(bv_sb2[:], bv)
                bv16_sb2 = const.tile([1, 256], BF16, name="bv16_sb")
                nc.gpsimd.dma_start(bv16_sb2[:], bv16)
                nc.gpsimd.dma_start(
                    v_sb[:].rearrange("p (b c) -> p b c", c=65)[:, :, 64:65],
                    vones,
                )
                return ones_sb2, bv_sb2, bv16_sb2

        # --- v projection machinery (dripped into phase 1 and attention) ---
        xv_tiles = {}
        vps_tiles = {}
        pending_v = []  # (t, step) with step NKT == bias matmul

        def emit_xv_loads(j):
            if j == 0:
                for kk in range(NKT):
                    xt = xvp.tile([128, 512], BF16, name=f"xv0_{kk}", tag="xv")
                    nc.gpsimd.dma_start(
                        xt[:], xT0b[128 * kk : 128 * (kk + 1), :]
                    )
                    xv_tiles[(0, kk)] = xt
                return
            for g in range(4):
                xt = xvp.tile([128, 2, 512], FP8, name=f"xv{j}_{g}", tag="xv")
                for s in range(2):
                    nc.gpsimd.dma_start(
                        xt[:, s, :],
                        xT[256 * g + 128 * s : 256 * g + 128 * (s + 1),
                           512 * j : 512 * (j + 1)],
                    )
                xv_tiles[(j, g)] = xt

        NDR = 4

        def queue_v_block(j, tb):
            nsteps = NKT + 1 if j == 0 else NDR + 1
            for step in range(nsteps):
                pending_v.append((4 * j + tb, step))

        def emit_v_steps(n):
            while n > 0 and pending_v:
                t, step = pending_v.pop(0)
                j, tb = t // 4, t % 4
                if step == 0:
                    vps_tiles[t] = psv.tile(
                        [128, 256], F32, name=f"vps{t}", tag="vps"
                    )
                vps = vps_tiles[t]
                nmm = NKT if j == 0 else NDR
                if step < nmm:
                    if j == 0:
                        nc.tensor.matmul(
                            vps[:],
                            xv_tiles[(0, step)][:, 128 * tb : 128 * (tb + 1)],
                            wvb_sb[step][:],
                            start=(step == 0),
                            stop=False,
                        )
                    else:
                        nc.tensor.matmul(
                            vps[:],
                            xv_tiles[(j, step)][:, :, 128 * tb : 128 * (tb + 1)],
                            wv_sb[step][:],
                            start=(step == 0),
                            stop=False,
                            perf_mode=DR,
                        )
                else:
                    nc.tensor.matmul(
                        vps[:], ones_sb[0:1, :],
                        bv_sb[:] if j == 0 else bv16_sb[:],
                        start=False, stop=True,
                    )
                    dst = v_sb[:, 260 * t : 260 * (t + 1)].rearrange(
                        "p (h c) -> p h c", c=65
                    )[:, :, 0:64]
                    srcv = vps_tiles.pop(t)[:].rearrange("p (h c) -> p h c", c=64)
                    if j == 0:
                        nc.vector.tensor_copy(dst, srcv)
                    else:
                        nc.vector.tensor_scalar_mul(dst, srcv, 1.0 / 16.0)
                n -= 1

        def flush_v_through(block):
            while pending_v and pending_v[0][0] <= block:
                emit_v_steps(1)

        # ---------------- phase 1: q/k projection ----------------
        # x tiles stay resident in SBUF (bf16): moving operand here, then
        # stationary operand of the v matmuls. Loads in consumption order.
        psv = ctx.enter_context(tc.tile_pool(name="psv", bufs=2, space="PSUM"))
        xvp = ctx.enter_context(tc.tile_pool(name="xvp", bufs=20))
        ppool = ctx.enter_context(tc.tile_pool(name="ppool", bufs=5))
        hoisted_p = {}
        with tc.tile_pool(name="psqk", bufs=6, space="PSUM") as psqk:
            for tb in range(4):
                queue_v_block(0, tb)
            for c4 in range(NQC):
                cs = slice(512 * c4, 512 * (c4 + 1))
                ps = [
                    psqk.tile([128, 512], F32, name=f"qkps{c4}_{m}", tag="qkps")
                    for m in range(4)
                ]
                if c4 == 0:
                    # bf16 chunk 0: early queries only attend early keys, so
                    # tokens 0-511 get a full-bf16 q/k/v path (fp8 logit noise
                    # does not average out over few-key softmaxes).
                    for kk in range(NKT):
                        t = wpool.tile([128, 512], BF16, name=f"wqkb{kk}")
                        nc.sync.dma_start(t[:], wqkb[128 * kk : 128 * (kk + 1), :])
                        wqkb_sb.append(t)
                        xt = xpool.tile([128, 512], BF16, name=f"xb{kk}", tag="xb")
                        eng = nc.sync if kk % 2 == 0 else nc.gpsimd
                        eng.dma_start(xt[:], xT0b[128 * kk : 128 * (kk + 1), :])
                        for m in range(4):
                            nc.tensor.matmul(
                                ps[m][:],
                                t[:, 128 * m : 128 * (m + 1)],
                                xt[:],
                                start=(kk == 0),
                                stop=(kk == NKT - 1),
                            )
                else:
                    for g in range(4):
                        if c4 == 1:
                            t = wpool.tile([128, 2, 512], FP8, name=f"wqk{g}")
                            for s in range(2):
                                nc.sync.dma_start(
                                    t[:, s, :],
                                    wqk[256 * g + 128 * s : 256 * g + 128 * (s + 1), :],
                                )
                            wqk_sb.append(t)
                        xt = xpool.tile([128, 2, 512], FP8, name=f"x{c4}_{g}", tag="x")
                        eng = nc.sync if g % 2 == 0 else nc.gpsimd
                        for s in range(2):
                            eng.dma_start(
                                xt[:, s, :],
                                xT[256 * g + 128 * s : 256 * g + 128 * (s + 1), cs],
                            )
                        x_tiles[(c4, g)] = xt
                        for m in range(4):
                            nc.tensor.matmul(
                                ps[m][:],
                                wqk_sb[g][:, :, 128 * m : 128 * (m + 1)],
                                xt[:],
                                start=(g == 0),
                                stop=(g == 3),
                                perf_mode=DR,
                            )
                        if c4 == 3:
                            emit_v_steps(5)
                for m in range(4):
                    dst = qT[m][:, cs] if m < 2 else kT[m - 2][:, cs]
                    nc.scalar.activation(
                        dst, ps[m][:], IDN,
                        bias=bqk_sb[:, m : m + 1],
                        scale=(1.0 if c4 == 0 else 1.0 / 16.0),
                    )
                if c4 == 1:
                    ones_sb, bv_sb, bv16_sb = emit_secondary_loads(1)
                    emit_xv_loads(0)
                if c4 == 0:
                    # hoist chunk (hp=0, j=0): scores + exp start here so the
                    # scalar engine ramps earlier; pv runs later in the
                    # main attention stream.
                    for hi in range(4):
                        lo = 128 * hi
                        w = 512 - lo
                        p = ppool.tile([128, 1024], BF16, name="p_pair")
                        for par in range(2):
                            sps = psqk.tile(
                                [128, 512], F32, name=f"hs{hi}_{par}", tag="qkps"
                            )
                            nc.tensor.matmul(
                                sps[:, lo:512],
                                kT[0][64 * par : 64 * (par + 1),
                                      128 * hi : 128 * (hi + 1)],
                                qT[0][64 * par : 64 * (par + 1), lo:512],
                                start=True,
                                stop=True,
                            )
                            nc.scalar.activation(
                                p[:, 512 * par + lo : 512 * (par + 1)],
                                sps[:, lo:512],
                                EXP,
                                scale=0.125,
                            )
                        off, w2, msk = 128 * hi, 128, mtri_sb
                        pv2 = p[:].rearrange("pt (a q) -> pt a q", q=512)[
                            :, :, off : off + w2
                        ]
                        mv2 = msk[:].rearrange(
                            "pt (a q) -> pt a q", a=1
                        ).broadcast_to([128, 2, w2])
                        nc.vector.tensor_mul(pv2, pv2, mv2)
                        hoisted_p[(0, 0, hi)] = p

        # secondary loads (needed from attention onwards)
        for hp in range(2):
            t = wpool.tile([128, 1024], BF16, name=f"wp{hp}")
            nc.gpsimd.dma_start(t[:], wp[128 * hp : 128 * (hp + 1), :])
            wp_sb.append(t)

        # ------- phase 2: attention (flat software-pipelined stream) -------
        # Items (hp, j, i) are processed in a single pipelined stream: the
        # score matmul pair + exp of item n issue together, the pv matmuls of
        # item n-2 follow, and each chunk's epilogue fires when its last pv
        # has issued. The v projection (hp=0) and the output projection
        # (hp=1) are drip-fed into the stream to fill tensor-engine slack
        # while exp paces the loop.
        with (
            tc.tile_pool(name="pss", bufs=2, space="PSUM") as pss,
            tc.tile_pool(name="psy", bufs=2, space="PSUM") as psy,
            tc.tile_pool(name="epool", bufs=2) as epool,
            tc.tile_pool(name="opool", bufs=4) as opool,
        ):
            o_ps_tiles = {}
            p_tiles = {}

            pending_proj = []
            proj_osb = {}

            def emit_proj_steps(n):
                # one (t, n2) half-block per step: 2 matmuls + evac; the
                # 512 KB store fires when both halves are done
                while n > 0 and pending_proj:
                    t, n2 = pending_proj.pop(0)
                    if n2 == 0:
                        proj_osb[t] = opool.tile(
                            [128, 1024], F32, name=f"po{t}", tag="po"
                        )
                    prps = psv.tile(
                        [128, 512], F32, name=f"prps{t}_{n2}", tag="vps"
                    )
                    for hp2 in range(2):
                        nc.tensor.matmul(
                            prps[:],
                            y_sb[hp2][:, 128 * t : 128 * (t + 1)],
                            wp_sb[hp2][:, 512 * n2 : 512 * (n2 + 1)],
                            start=(hp2 == 0),
                            stop=(hp2 == 1),
                        )
                    if n2 == 0:
                        nc.vector.tensor_copy(
                            proj_osb[t][:, 512 * n2 : 512 * (n2 + 1)], prps[:]
                        )
                    else:
                        nc.scalar.copy(
                            proj_osb[t][:, 512 * n2 : 512 * (n2 + 1)], prps[:]
                        )
                    if n2 == 1:
                        nc.sync.dma_start(
                            out[128 * t : 128 * (t + 1), :], proj_osb.pop(t)[:]
                        )
                    n -= 1

            def queue_proj(j):
                for t in range(4 * j, 4 * j + 4):
                    for n2 in range(2):
                        pending_proj.append((t, n2))

            def emit_s_exp(hp, j, i):
                d = i - 4 * j
                lo = min(max(0, d) * 128, 384)
                s_pair = pss.tile([128, 1024], F32, name="s_pair")
                for par in range(2):
                    nc.tensor.matmul(
                        s_pair[:, 512 * par + lo : 512 * (par + 1)],
                        kT[hp][64 * par : 64 * (par + 1), 128 * i : 128 * (i + 1)],
                        qT[hp][
                            64 * par : 64 * (par + 1),
                            512 * j + lo : 512 * (j + 1),
                        ],
                        start=True,
                        stop=True,
                    )
                p = ppool.tile([128, 1024], BF16, name="p_pair")
                nc.scalar.activation(
                    p[:, lo:1024], s_pair[:, lo:1024], EXP, scale=0.125
                )
                # causal masks: multiplicative 0/1 on p (both heads in one
                # op via a strided view + free-dim broadcast of the mask)
                if d >= 0:
                    off, w, msk = 128 * d, 128, mtri_sb
                    pv2 = p[:].rearrange("pt (a q) -> pt a q", q=512)[
                        :, :, off : off + w
                    ]
                    mv2 = msk[:].rearrange("pt (a q) -> pt a q", a=1).broadcast_to(
                        [128, 2, w]
                    )
                    nc.vector.tensor_mul(pv2, pv2, mv2)
                p_tiles[(hp, j, i)] = p

            def emit_pv(hp, j, i):
                nk = 4 * j + 4
                lo = min(max(0, i - 4 * j) * 128, 384)
                if hp == 0:
                    flush_v_through(i)
                if i == 0:
                    o_ps_tiles[(hp, j)] = [
                        psy.tile([65, 512], F32, name=f"o_ps{hp}{j}{par}", tag="o_ps")
                        for par in range(2)
                    ]
                o_ps = o_ps_tiles[(hp, j)]
                p = p_tiles.pop((hp, j, i))
                for par in range(2):
                    h = 2 * hp + par
                    vcol = 260 * i + 65 * h
                    nc.tensor.matmul(
                        o_ps[par][:, lo:512],
                        v_sb[:, vcol : vcol + 65],
                        p[:, 512 * par + lo : 512 * (par + 1)],
                        start=(i == 0),
                        stop=(i == nk - 1),
                    )
                if i == nk - 1:
                    emit_epilogue(hp, j)

            def emit_epilogue(hp, j):
                # denominator (row 64) -> broadcast -> fast reciprocal ->
                # fused normalize; decoupled from the main stream once the
                # [65, 512] psum is evacuated (in bf16) to SBUF.
                jc = slice(512 * j, 512 * (j + 1))
                o_ps = o_ps_tiles.pop((hp, j))
                for par in range(2):
                    o_sb = epool.tile([65, 512], BF16, name=f"oe{par}", tag="o_sb")
                    nc.vector.tensor_copy(o_sb[:], o_ps[par][:])
                    bc = psy.tile([64, 512], F32, name=f"bc{par}", tag="o_ps")
                    nc.tensor.matmul(
                        bc[:], ones_sb[64:65, 0:64], o_sb[64:65, :],
                        start=True, stop=True,
                    )
                    r_sb = epool.tile([64, 512], F32, name=f"r_sb{par}", tag="r_sb")
                    nc.vector.reciprocal_approx_fast(out=r_sb[:], in_=bc[:])
                    nc.vector.tensor_mul(
                        y_sb[hp][64 * par : 64 * (par + 1), jc],
                        o_sb[0:64, :],
                        r_sb[:],
                    )
                if hp == 1:
                    queue_proj(j)

            emit_xv_loads(1)

            items = [
                (0, j, i) for j in range(NQC) for i in range(4 * j + 4)
            ] + [
                (1, j, i) for j in (1, 0, 2, 3) for i in range(4 * j + 4)
            ]
            LAG = 3
            for n, (hp, j, i) in enumerate(items):
                if (hp, j, i) in hoisted_p:
                    p_tiles[(hp, j, i)] = hoisted_p.pop((hp, j, i))
                else:
                    emit_s_exp(hp, j, i)
                # drip-fed side work on the tensor engine:
                if hp == 0 and j >= 1:
                    if i == 0:
                        for tb in range(4):
                            queue_v_block(j, tb)
                    emit_v_steps(2)
                if hp == 0 and i == 2 and j < 2:
                    emit_xv_loads(j + 2)
                if hp == 1 and i == 3:
                    emit_proj_steps(8)
                if n >= LAG:
                    emit_pv(*items[n - LAG])
            for n in range(len(items) - LAG, len(items)):
                emit_pv(*items[n])
            emit_proj_steps(len(pending_proj))

    nc.compile()
    return nc


def _get_nc():
    if "nc" not in _CACHE:
        _CACHE["nc"] = _build()
    return _CACHE["nc"]


def _prep_inputs(x, W_qkv, b_qkv, W_proj, b_proj):
    """Build the 8 per-core input maps."""
    x = np.asarray(x, dtype=np.float32)
    W_qkv = np.asarray(W_qkv, dtype=np.float32)
    b_qkv = np.asarray(b_qkv, dtype=np.float32)
    W_proj = np.asarray(W_proj, dtype=np.float32)

    ones = np.ones((128, 128), dtype=BF16NP)
    vones = np.ones((128, 64, 1), dtype=BF16NP)
    mtri = (np.arange(128)[:, None] <= np.arange(128)[None, :]).astype(BF16NP)

    xT_aug = {}
    xT0_bf = {}
    for b in range(B):
        xT_aug[b] = _fp8(x[b].T)
        xT0_bf[b] = _bf16(x[b].T[:, 0:512])

    in_maps = []
    for c in range(N_CORES):
        b = c // 4
        g = c % 4
        hs = slice(256 * g, 256 * (g + 1))
        wqk_c = np.concatenate(
            [W_qkv[:, 0:1024][:, hs], W_qkv[:, 1024:2048][:, hs]], axis=1
        )
        wv_c = W_qkv[:, 2048:3072][:, hs]
        bqk_h = np.stack(
            [b_qkv[0:1024][hs], b_qkv[1024:2048][hs]]
        )  # [2, 256] -> m blocks of 128
        bqk_m = np.concatenate([bqk_h[0], bqk_h[1]]).reshape(4, 128).T.copy()
        bv_h = _bf16(16.0 * b_qkv[2048:3072][hs].reshape(1, 256))
        wp_c = _bf16(W_proj[hs, :])
        in_maps.append(
            {
                "xT": xT_aug[b],
                "xT0b": xT0_bf[b],
                "wqk": _fp8(16.0 * wqk_c),
                "wqkb": _bf16(wqk_c),
                "wv": _fp8(16.0 * wv_c),
                "wvb": _bf16(wv_c),
                "wp": wp_c,
                "ones": ones,
                "bqk": np.ascontiguousarray(bqk_m),
                "bv": _bf16(b_qkv[2048:3072][hs].reshape(1, 256)),
                "bv16": bv_h,
                "vones": vones,
                "mtri": np.ascontiguousarray(mtri),
            }
        )
    return in_maps


def kernel(x, W_qkv, b_qkv, W_proj, b_proj, K=None, _trace=False):
    from concourse.bass_utils import run_bass_kernel_spmd

    in_maps = _prep_inputs(x, W_qkv, b_qkv, W_proj, b_proj)
    nc = _get_nc()
    res = run_bass_kernel_spmd(
        nc, in_maps, core_ids=list(range(N_CORES)), trace=_trace
    )
    parts = [res.results[c]["out"] for c in range(N_CORES)]
    b_proj = np.asarray(b_proj, dtype=np.float32)
    y = np.empty((B, SEQ, C), dtype=np.float32)
    for b in range(B):
        y[b] = parts[4 * b] + parts[4 * b + 1] + parts[4 * b + 2] + parts[4 * b + 3]
        y[b] += b_proj
    if _trace:
        _CACHE["last_exec_time_ns"] = res.exec_time_ns
        _CACHE["last_results"] = res
    return y
